# revision 40
# baseline (speedup 1.0000x reference)
"""Causal attention (nn_CausalAttention) TRN2 Bass kernel — v2.

Tensor-parallel over the 16 heads -> 2 heads per NeuronCore. Per core:
  - qkv projection computed transposed (col-major): M_s = W_s^T @ x^T on the
    PE; evictions (DVE, bias add, bf16 out) scatter stride-3 into qT/kT/vT,
    reproducing the reference's raw-memory reshape(3,B,T,HS,NH) semantics.
  - x^T staged in DRAM as [128, (dc t)] so each proj p-tile is ONE DMA.
  - causal attention with scores transposed (tk on partitions), all
    attention operands bf16 on the PE (1 cyc/row at any width), exact
    causal trimming with the 4 diagonal chunks packed into two psum
    tiles, exp on ScalarE only (scale folded), triu masks on Pool.
  - A@V accumulated in PSUM with an appended ones-column producing
    softmax row sums; epilogue transposes ctx via PE (bf16), DVE
    reciprocal + normalize, one output DMA per 512-token tile.
Emission is software-pipelined (each unit's AV is emitted one unit late)
with proj s-blocks / vext transpose packs doled out between attention
units; the ASAP tile scheduler finalizes per-engine order.
"""

import os
import sys
from collections import deque

import numpy as np

for _p in ("/opt/trn_rl_repo", "/root/.axon_site/_ro/trn_rl_repo"):
    if os.path.isdir(_p) and _p not in sys.path:
        sys.path.insert(0, _p)

import concourse.bass as bass  # noqa: E402
import concourse.mybir as mybir  # noqa: E402
import concourse.tile as tile  # noqa: E402
from concourse import bacc  # noqa: E402
from concourse.bass_utils import run_bass_kernel_spmd  # noqa: E402
from concourse.masks import make_identity  # noqa: E402

f32 = mybir.dt.float32
f32r = mybir.dt.float32r
bf16 = mybir.dt.bfloat16
AF = mybir.ActivationFunctionType

T, D, NH, HS = 4096, 1024, 16, 64
SCALE = HS ** -0.5  # 0.125
i16 = mybir.dt.int16
# bf16-space Schraudolph fast-exp: exp(s*SCALE) ~= bitcast_bf16(A16*s + B16)
FEXP_A = 128.0 / 0.6931471805599453 * SCALE  # 2^7/ln2 * SCALE
FEXP_B = 16256.0 - 5.58  # 127*2^7 minus mean-error centering
# offload clean-unit exps to DVE fast-exp on late (Act-bound) tiles
FAST_I0 = int(os.environ.get("KFAST_I0", "5"))
FAST_MOD = int(os.environ.get("KFAST_MOD", "2"))
KLAG = int(os.environ.get("KLAG", "2"))  # units of AV emission lag
NCORES = 8
TQ = 512
NTQ = T // TQ  # 8
TK = 128
NTK = T // TK  # 32

# (u_lo, u_hi, dst, t0): M_s[:, u] for u in [u_lo,u_hi) -> dst col 3*(u-u_lo)+t0
SPLITS = {
    0: [(0, 1366, "q", 0), (1366, 2731, "k", 2), (2731, 4096, "v", 1)],
    1: [(0, 1365, "q", 1), (1365, 2731, "k", 0), (2731, 4096, "v", 2)],
    2: [(0, 1365, "q", 2), (1365, 2730, "k", 1), (2730, 4096, "v", 0)],
}

# sub-tile column ranges per destination (t-space); boundaries align with
# projection u-tile boundaries (r = 3u+s, r-boundaries multiples of 1536)
SUBS = {
    "q": [(0, 1536), (1536, 3072), (3072, 4096)],
    "k": [(0, 512), (512, 2048), (2048, 3584), (3584, 4096)],
    "v": [(0, 1024), (1024, 2560), (2560, 4096)],
}
BASE_R = {"q": 0, "k": 4096, "v": 8192}
# vext storage groups (aligned to v sub-tiles): chunk ranges
VE_GROUPS = [(0, 8), (8, 20), (20, 32)]

# proj s-blocks + vext packs + xt prefetches due before attention tile i
DUE = {
    1: [("ps", 3, 0), ("ps", 3, 1), ("xt", 6), ("ps", 3, 2), ("a2", 1)],
    2: [("ps", 6, 0), ("ps", 6, 1), ("xt", 1), ("ps", 6, 2), ("a2", 2)],
    3: [("ps", 1, 0), ("ps", 1, 1), ("xt", 4), ("ps", 1, 2), ("a2", 3)],
    4: [("ps", 4, 0), ("ps", 4, 1), ("xt", 7), ("ps", 4, 2), ("a2", 4)],
    5: [("ps", 7, 0), ("ps", 7, 1), ("ps", 7, 2), ("a2", 5)],
    6: [("a2", 6)],
    7: [("a2", 7)],
    8: [],
}


def _pad3(w):
    return ((w + 2) // 3) * 3


def _sub_for(dst, t):
    for j, (lo, hi) in enumerate(SUBS[dst]):
        if lo <= t < hi:
            return j
    raise ValueError((dst, t))


class Rep:
    def __init__(self, nc, rep, consts, bigs, pools, const_dmas=None):
        self.nc = nc
        self.rep = rep
        (self.w_all, self.b_sb, self.mask_sb, self.ident) = consts
        (self.qs, self.ks, self.vs, self.ves) = bigs
        (self.xt_pool, self.sc_ps, self.ctx_ps, self.exp_pool, self.cs_pool,
         self.out_pool, self.rec_pool) = pools
        self.const_dmas = const_dmas  # emitted by rep 0 after first xt split
        self.megas = {}

    # ---------------- DMA / proj ----------------

    def xt_load(self, p, nsplit=1, interleave=None):
        nc = self.nc
        xt_d = self.nc._io["xt"]  # [128, 8*T], layout (dc, t)
        t0 = self.xt_pool.tile([128, 8 * TQ], bf16, tag="xt",
                               name=f"xt_{self.rep}_{p}")
        src = xt_d.rearrange("d (dc t) -> d dc t", t=T)[:, :, TQ * p:TQ * (p + 1)]
        dst = t0[:].rearrange("d (dc t) -> d dc t", t=TQ)
        step = 8 // nsplit
        for s0 in range(0, 8, step):
            nc.sync.dma_start(out=dst[:, s0:s0 + step, :],
                              in_=src[:, s0:s0 + step, :])
            if s0 == 0 and interleave is not None:
                interleave()
        self.megas[p] = t0

    def proj_s(self, p, s):
        nc = self.nc
        mega = self.megas[p][:].rearrange("d (dc t) -> d dc t", t=TQ)
        ps = self.sc_ps.tile([128, 1024], f32, tag="sc",
                             name=f"proj_{self.rep}_{p}_{s}")
        for dc in range(8):
            g = s * 8 + dc
            nc.tensor.matmul(
                ps[:, 0:TQ],
                self.w_all[:, g * 128:(g + 1) * 128],
                mega[:, dc, :],
                start=(dc == 0),
                stop=(dc == 7),
            )
        u0, u1 = TQ * p, TQ * (p + 1)
        for lo, hi, dst, t0 in SPLITS[s]:
            a0, a1 = max(lo, u0), min(hi, u1)
            if a0 >= a1:
                continue
            r0 = 3 * a0 + s
            t_g = r0 - BASE_R[dst]
            j = _sub_for(dst, t_g)
            sub = {"q": self.qs, "k": self.ks, "v": self.vs}[dst][j]
            t_lo = SUBS[dst][j][0]
            tl = t_g - t_lo
            a_idx, e_idx = tl // 3, tl % 3
            view = sub[:].rearrange("p (a e) -> p a e", e=3)[
                :, a_idx:a_idx + (a1 - a0), e_idx
            ]
            nc.vector.tensor_scalar_add(
                view, ps[:, a0 - u0:a1 - u0], self.b_sb[:, s:s + 1]
            )

    def a2_pack(self, k):
        """Transpose vT chunks 4k..4k+3 into vext layout (ones cols preset)."""
        nc = self.nc
        gi = next(g for g, (c0, c1) in enumerate(VE_GROUPS)
                  if c0 <= 4 * k < c1)
        c0g = VE_GROUPS[gi][0]
        vlo = SUBS["v"][gi][0]
        vsub = self.vs[gi]
        trp = self.sc_ps.tile([128, 1024], f32, tag="sc",
                              name=f"vtr_{self.rep}_{k}")
        for idx in range(4):
            c = 4 * k + idx
            toff = 128 * c - vlo
            nc.tensor.transpose(
                trp[:, 128 * idx:128 * (idx + 1)],
                vsub[:, toff:toff + 128],
                self.ident[:],
            )
        for h in (0, 1):
            src = trp[:, 0:512].rearrange("p (c f) -> p c f", f=128)[
                :, :, 64 * h:64 * h + 64
            ]
            dst = self.ves[gi][h][:].rearrange("p (c e) -> p c e", e=65)[
                :, 4 * k - c0g:4 * k - c0g + 4, 0:64
            ]
            nc.vector.tensor_copy(dst, src)

    # ---------------- attention helpers ----------------

    def _kT(self, c):
        t = 128 * c
        j = _sub_for("k", t)
        lo = SUBS["k"][j][0]
        return self.ks[j][:, t - lo:t - lo + 128]

    def _q(self, i, h, vs=0):
        j = _sub_for("q", TQ * i)
        qlo = SUBS["q"][j][0]
        return self.qs[j][64 * h:64 * h + 64,
                          TQ * i - qlo + vs:TQ * (i + 1) - qlo]

    def _vext(self, c, h):
        for gi, (c0, c1) in enumerate(VE_GROUPS):
            if c0 <= c < c1:
                return self.ves[gi][h][:, 65 * (c - c0):65 * (c - c0) + 65]
        raise ValueError(c)

    # units: ("D1", i, h) = diag chunks 4i,4i+1 ; ("D2", i, h) = 4i+2,4i+3
    #        ("C", i, h, g) = clean chunks 2g, 2g+1
    def unit_chunks(self, u):
        kind, i, h = u[0], u[1], u[2]
        if kind == "D1":
            # (chunk, psum col offset, valid tq start)
            return [(4 * i, 0, 0), (4 * i + 1, 512, 128)]
        if kind == "D2":
            return [(4 * i + 2, 0, 256), (4 * i + 3, 256, 384)]
        g = u[3]
        return [(2 * g, 0, 0), (2 * g + 1, 512, 0)]

    def emit_scores_exp(self, u):
        nc = self.nc
        kind, i, h = u[0], u[1], u[2]
        chunks = self.unit_chunks(u)
        name = f"{kind}_{self.rep}_{i}_{h}" + (f"_{u[3]}" if kind == "C" else "")
        sct = self.sc_ps.tile([128, 1024], f32, tag="sc", name=f"sc_{name}")
        for c, off, vs in chunks:
            nc.tensor.matmul(
                sct[:, off:off + TQ - vs],
                self._kT(c)[64 * h:64 * h + 64, :],
                self._q(i, h, vs),
                start=True,
                stop=True,
            )
        ext = self.exp_pool.tile([128, 1024], bf16, tag="exp",
                                 name=f"exp_{name}")
        last_c, last_off, last_vs = chunks[-1]
        width = last_off + TQ - last_vs
        fast = (kind == "C" and i >= FAST_I0 and u[3] % FAST_MOD == FAST_MOD - 1)
        if fast:
            nc.vector.tensor_scalar(
                ext[:, 0:width].bitcast(i16), sct[:, 0:width], FEXP_A, FEXP_B,
                op0=mybir.AluOpType.mult, op1=mybir.AluOpType.add,
            )
        else:
            nc.scalar.activation(ext[:, 0:width], sct[:, 0:width], AF.Exp,
                                 scale=SCALE)
        if kind in ("D1", "D2"):
            for c, off, vs in chunks:
                nc.gpsimd.tensor_mul(
                    ext[:, off:off + 128], ext[:, off:off + 128],
                    self.mask_sb[:],
                )
        u_ctx = (u, ext)
        return u_ctx

    def emit_av(self, u_ctx, ctx_tiles, is_first, is_last):
        nc = self.nc
        u, ext = u_ctx
        kind, i, h = u[0], u[1], u[2]
        chunks = self.unit_chunks(u)
        for idx, (c, off, vs) in enumerate(chunks):
            first = is_first and idx == 0
            last = is_last and idx == len(chunks) - 1
            nc.tensor.matmul(
                ctx_tiles[h][:, vs:TQ],
                self._vext(c, h),
                ext[:, off:off + TQ - vs],
                start=first,
                stop=last,
            )

    def epilogue(self, i, h, ctx_tiles, osb):
        nc = self.nc
        cs = self.cs_pool.tile([65, TQ], f32, tag="cs",
                               name=f"cs_{self.rep}_{i}_{h}")
        nc.vector.tensor_copy(cs[:], ctx_tiles[h][:])
        trp = self.sc_ps.tile([128, 1024], f32, tag="sc",
                              name=f"ctr_{self.rep}_{i}_{h}")
        for k4 in range(4):
            nc.tensor.transpose(
                trp[:, 65 * k4:65 * (k4 + 1)],
                cs[:, 128 * k4:128 * (k4 + 1)],
                self.ident[0:65, 0:65],
            )
        rec = self.rec_pool.tile([128, 4], f32, tag="rec",
                                 name=f"rec_{self.rep}_{i}_{h}")
        nc.vector.reciprocal(
            rec[:], trp[:, 0:260].rearrange("p (k e) -> p k e", e=65)[:, :, 64]
        )
        for k4 in range(4):
            nc.vector.tensor_scalar_mul(
                osb[:, 128 * k4 + 64 * h:128 * k4 + 64 * h + 64],
                trp[:, 65 * k4:65 * k4 + 64],
                rec[:, k4:k4 + 1],
            )

    def out_dma(self, i, osb):
        nc = self.nc
        out_d = nc._io["out"]
        dst = out_d[TQ * i:TQ * (i + 1), :].rearrange("(k p) f -> p k f", p=128)
        nc.sync.dma_start(
            out=dst, in_=osb[:].rearrange("p (k f) -> p k f", f=128)
        )

    def do_item(self, it):
        if it[0] == "ps":
            self.proj_s(it[1], it[2])
        elif it[0] == "a2":
            self.a2_pack(it[1])
        elif it[0] == "xt":
            self.xt_load(it[1])

    # ---------------- orchestration ----------------

    def units_for(self, i):
        # D1 first (full-width start, masks early); one clean group between
        # D1 and D2 hides the diagonal exp latency; remaining cleans follow.
        units = [("D1", i, 0), ("D1", i, 1)]
        if i > 0:
            units += [("C", i, 0, 0), ("C", i, 1, 0)]
        units += [("D2", i, 0), ("D2", i, 1)]
        for g in range(1, 2 * i):
            units.append(("C", i, 0, g))
            units.append(("C", i, 1, g))
        return units

    def emit(self):
        self.xt_load(2, nsplit=8, interleave=self.const_dmas)
        for p in (5, 0):
            self.xt_load(p)
        for it in ([("ps", 2, s) for s in range(3)]
                   + [("xt", 3)]
                   + [("ps", 5, s) for s in range(3)]
                   + [("a2", 0)]
                   + [("ps", 0, s) for s in range(3)]):
            self.do_item(it)

        pending = deque()  # (u_ctx, ctx_tiles, is_first, is_last, epi_info)
        for i in range(NTQ):
            fillers = deque(DUE[i + 1])
            units = self.units_for(i)
            nu = len(units)
            # ctx psum tiles for this tile
            new_ctx = [
                self.ctx_ps.tile([65, TQ], f32, tag="ctx",
                                 name=f"ctx_{self.rep}_{i}_{hh}")
                for hh in (0, 1)
            ]
            new_osb = self.out_pool.tile([128, 512], f32, tag="osb",
                                         name=f"osb_{self.rep}_{i}")
            # last unit index per head (for stop flag / epilogue)
            last_of_h = {0: nu - 2, 1: nu - 1}
            first_of_h = {0: 0, 1: 1}
            stride = max(1, nu // (len(fillers) + 1)) if fillers else nu
            for ui, u in enumerate(units):
                u_ctx = self.emit_scores_exp(u)
                if len(pending) >= KLAG:
                    self.flush_av(pending.popleft())
                h = u[2]
                pending.append(
                    (u_ctx, new_ctx, ui == first_of_h[h],
                     ui == last_of_h[h],
                     (u[1], h, new_osb) if ui == last_of_h[h] else None)
                )
                if fillers and ui % stride == stride - 1:
                    self.do_item(fillers.popleft())
            while fillers:
                self.do_item(fillers.popleft())
        while pending:
            self.flush_av(pending.popleft())

    def flush_av(self, pending):
        u_ctx, ctx_tiles, is_first, is_last, epi = pending
        self.emit_av(u_ctx, ctx_tiles, is_first, is_last)
        if epi is not None:
            i, h, osb = epi
            self.epilogue(i, h, ctx_tiles, osb)
            if h == 1:
                self.out_dma(i, osb)


def _build_program(reps=1, trace_sim=False):
    nc = bacc.Bacc(
        "TRN2", target_bir_lowering=False, debug=False, num_devices=NCORES
    )
    nc._io = {
        "xt": nc.dram_tensor("xt", [128, 8 * T], bf16, kind="ExternalInput").ap(),
        "w": nc.dram_tensor("w", [128, 3 * 1024], bf16, kind="ExternalInput").ap(),
        "b": nc.dram_tensor("b", [128, 3], f32, kind="ExternalInput").ap(),
        "m": nc.dram_tensor("m", [128, 128], bf16, kind="ExternalInput").ap(),
        "out": nc.dram_tensor("out", [T, 128], f32, kind="ExternalOutput").ap(),
    }

    with tile.TileContext(nc, trace_sim=trace_sim) as tc:
        with (
            tc.tile_pool(name="const", bufs=1) as const_pool,
            tc.tile_pool(name="big", bufs=1) as big_pool,
            tc.tile_pool(name="xtp", bufs=8) as xt_pool,
            tc.tile_pool(name="scps", bufs=3, space="PSUM") as sc_ps,
            tc.tile_pool(name="ctxps", bufs=2, space="PSUM") as ctx_ps,
            tc.tile_pool(name="expp", bufs=6) as exp_pool,
            tc.tile_pool(name="csp", bufs=4) as cs_pool,
            tc.tile_pool(name="outp", bufs=2) as out_pool,
            tc.tile_pool(name="recp", bufs=4) as rec_pool,
        ):
            w_all = const_pool.tile([128, 3 * 8 * 128], bf16, tag="w_all")
            b_sb = const_pool.tile([128, 3], f32, tag="b_sb")
            mask_sb = const_pool.tile([128, 128], bf16, tag="mask")
            ident = const_pool.tile([128, 128], f32, tag="ident")
            make_identity(nc, ident[:])

            def const_dmas():
                for s in range(3):
                    nc.sync.dma_start(
                        out=w_all[:, s * 1024:(s + 1) * 1024],
                        in_=nc._io["w"][:, s * 1024:(s + 1) * 1024],
                    )
                nc.sync.dma_start(out=b_sb[:], in_=nc._io["b"][:])
                nc.sync.dma_start(out=mask_sb[:], in_=nc._io["m"][:])

            big_sets = []
            for par in (0, 1):
                qs = [
                    big_pool.tile([128, _pad3(hi - lo)], bf16,
                                  tag=f"q{j}_{par}", name=f"qT{j}_{par}")
                    for j, (lo, hi) in enumerate(SUBS["q"])
                ]
                ks = [
                    big_pool.tile([128, _pad3(hi - lo)], bf16,
                                  tag=f"k{j}_{par}", name=f"kT{j}_{par}")
                    for j, (lo, hi) in enumerate(SUBS["k"])
                ]
                vs = [
                    big_pool.tile([128, _pad3(hi - lo)], f32,
                                  tag=f"v{j}_{par}", name=f"vT{j}_{par}")
                    for j, (lo, hi) in enumerate(SUBS["v"])
                ]
                ves = [
                    [
                        big_pool.tile([128, 65 * (c1 - c0)], bf16,
                                      tag=f"ve{gi}{h}_{par}",
                                      name=f"vext{gi}{h}_{par}")
                        for h in (0, 1)
                    ]
                    for gi, (c0, c1) in enumerate(VE_GROUPS)
                ]
                # ones columns: preset once (never overwritten by a2 copies)
                for gi, (c0, c1) in enumerate(VE_GROUPS):
                    for h in (0, 1):
                        ones_view = ves[gi][h][:].rearrange(
                            "p (c e) -> p c e", e=65
                        )[:, :, 64]
                        nc.vector.memset(ones_view, 1.0)
                big_sets.append((qs, ks, vs, ves))

            consts = (w_all, b_sb, mask_sb, ident)
            pools = (xt_pool, sc_ps, ctx_ps, exp_pool, cs_pool, out_pool,
                     rec_pool)
            for rep in range(reps):
                Rep(nc, rep, consts, big_sets[rep % 2], pools,
                    const_dmas if rep == 0 else None).emit()

    nc.compile()
    return nc


def _round_f32r(x: np.ndarray) -> np.ndarray:
    """Round fp32 to fp32r (11-bit mantissa, RNE) — matches TRN2 hardware."""
    xi = np.ascontiguousarray(x).view(np.uint32)
    keep = xi & np.uint32(0xFFFFF000)
    rem = xi & np.uint32(0xFFF)
    half = np.uint32(0x800)
    lowbit = np.uint32(0x1000)
    up = keep + lowbit
    use_up = (rem > half) | ((rem == half) & ((keep & lowbit) != 0))
    return np.where(use_up, up, keep).astype(np.uint32).view(np.float32)


_NC = None


def _get_program():
    global _NC
    if _NC is None:
        _NC = _build_program(
            reps=int(os.environ.get("KREPS", "1")),
            trace_sim=bool(int(os.environ.get("KTRACE", "0"))),
        )
    return _NC


def prepare_inputs(x, Wqkv, bqkv):
    x = np.asarray(x, dtype=np.float32)
    Wqkv = np.asarray(Wqkv, dtype=np.float32)
    bqkv = np.asarray(bqkv, dtype=np.float32)
    npbf = mybir.dt.np(bf16)
    # xt2[d', dc*T + t] = x[t, dc*128+d']
    xt2 = np.ascontiguousarray(
        x.reshape(T, 8, 128).transpose(2, 1, 0).reshape(128, 8 * T)
    ).astype(npbf)
    mask = np.triu(np.ones((128, 128), np.float32)).astype(npbf)  # keep tk<=tq
    in_maps = []
    for c in range(NCORES):
        h0, h1 = 2 * c, 2 * c + 1
        cols = np.concatenate([np.arange(HS) * NH + h0, np.arange(HS) * NH + h1])
        w_c = np.stack([Wqkv[:, s * D + cols] for s in range(3)])  # (3,1024,128)
        # w2[d, s*1024 + dc*128 + c] = w_c[s, dc*128+d, c]
        w2 = np.ascontiguousarray(
            w_c.reshape(3, 8, 128, 128).transpose(2, 0, 1, 3).reshape(128, 3072)
        ).astype(npbf)
        b_c = np.stack([bqkv[s * D + cols] for s in range(3)], axis=1)  # (128,3)
        in_maps.append(
            {
                "xt": xt2,
                "w": w2,
                "b": np.ascontiguousarray(b_c),
                "m": mask,
            }
        )
    return in_maps


def kernel(x, Wqkv, bqkv):
    nc = _get_program()
    in_maps = prepare_inputs(x, Wqkv, bqkv)
    res = run_bass_kernel_spmd(nc, in_maps, list(range(NCORES)))
    out = np.empty((1, T, D), np.float32)
    for c in range(NCORES):
        out[0, :, 128 * c:128 * (c + 1)] = res.results[c]["out"]
    return out
